# revision 10
# baseline (speedup 1.0000x reference)
"""Trainium2 Bass kernel for KAttentionalPropagation.

Shapes (hardcoded): B=4, D=256, H=4 heads (HD=64), N=M=2048.
Sharding: 8 cores = (batch b, query half s). Each core handles 1024 queries of
one batch against all 2048 keys. Zero cross-core communication.

Math per core (derived from the reference):
  q = Wq x + bq ; k = Wk s + bk ; v = Wv s + bv       (channels permuted head-major)
  scoresT[m,n] = (k_h^T q_h)[m,n]                      (keys on partitions)
  e = exp(scoresT * wmask)   with wmask = mask^T * weight/8   (host-folded)
  umsg[c,n], sumexp[n] = (vT | ones)^T e               (ones col -> denominator)
  msg = umsg / sumexp        (GPSIMD partition-broadcast + DVE mul)
  mm = Wm msg + bm ; h1 = relu(W1' [x; mm] + b1') ; out = W2 h1 + b2
  (BN folded into W1'/b1' on host.)
"""

import os
import numpy as np

import concourse.bass as bass
import concourse.bacc as bacc
import concourse.mybir as mybir
import concourse.tile as tile
from concourse.bass_utils import run_bass_kernel_spmd

F32 = mybir.dt.float32
AF = mybir.ActivationFunctionType

B, D, H, N, M = 4, 256, 4, 2048, 2048
HD = D // H          # 64
NC = N // 2          # queries per core = 1024
P = 128
N_CORES = 8

# dtype knobs (flipped for perf experiments)
MASK_DT = F32
EXP_DT = F32

_cached = {}


def build_program(zero_bias=False):
    nc = bacc.Bacc("TRN2", target_bir_lowering=False, debug=False, num_devices=N_CORES)

    x_d = nc.declare_dram_parameter("x_sl", [D, NC], F32, isOutput=False)
    src_d = nc.declare_dram_parameter("src", [D, M], F32, isOutput=False)
    wm_d = nc.declare_dram_parameter("wmask", [M, NC], MASK_DT, isOutput=False)
    wqT_d = nc.declare_dram_parameter("wqT", [D, D], F32, isOutput=False)
    wkT_d = nc.declare_dram_parameter("wkT", [D, D], F32, isOutput=False)
    wvT_d = nc.declare_dram_parameter("wvT", [D, 4 * (HD + 1)], F32, isOutput=False)
    vrow_d = nc.declare_dram_parameter("vrow", [1, 4 * (HD + 1)], F32, isOutput=False)
    wmT_d = nc.declare_dram_parameter("wmT", [D, D], F32, isOutput=False)
    w1T_d = nc.declare_dram_parameter("w1T", [2 * D, 2 * D], F32, isOutput=False)
    w2T_d = nc.declare_dram_parameter("w2T", [2 * D, D], F32, isOutput=False)
    bq_d = nc.declare_dram_parameter("bq2", [P, 2], F32, isOutput=False)
    bk_d = nc.declare_dram_parameter("bk2", [P, 2], F32, isOutput=False)
    bm_d = nc.declare_dram_parameter("bm2", [P, 2], F32, isOutput=False)
    b1_d = nc.declare_dram_parameter("b1p4", [P, 4], F32, isOutput=False)
    b2_d = nc.declare_dram_parameter("b22", [P, 2], F32, isOutput=False)
    out_d = nc.declare_dram_parameter("out", [D, NC], F32, isOutput=True)

    with tile.TileContext(nc) as tc:
        with (
            tc.tile_pool(name="const", bufs=1) as cpool,
            tc.tile_pool(name="persist", bufs=1) as ppool,
            tc.tile_pool(name="wm", bufs=3) as wmpool,
            tc.tile_pool(name="mk", bufs=3) as mkpool,
            tc.tile_pool(name="ex", bufs=3) as expool,
            tc.tile_pool(name="sm", bufs=2) as smpool,
            tc.tile_pool(name="ot", bufs=4) as otpool,
            tc.tile_pool(name="psb", bufs=2, space=bass.MemorySpace.PSUM) as psb,
            tc.tile_pool(name="psa", bufs=4, space=bass.MemorySpace.PSUM) as psa,
        ):
            # ---- constants / weights to SBUF ----
            def ctile(shape, tag, src_ap):
                t = cpool.tile(shape, F32, tag=tag, name=tag)
                nc.sync.dma_start(t[:], src_ap)
                return t

            wqT = [ctile([P, D], f"wq{i}", wqT_d[i * P:(i + 1) * P, :]) for i in range(2)]
            wkT = [ctile([P, D], f"wk{i}", wkT_d[i * P:(i + 1) * P, :]) for i in range(2)]
            wvT = [ctile([P, 4 * (HD + 1)], f"wv{i}", wvT_d[i * P:(i + 1) * P, :]) for i in range(2)]
            wmT = [ctile([P, D], f"wm{i}", wmT_d[i * P:(i + 1) * P, :]) for i in range(2)]
            w1T = [ctile([P, 2 * D], f"w1{i}", w1T_d[i * P:(i + 1) * P, :]) for i in range(4)]
            w2T = [ctile([P, D], f"w2{i}", w2T_d[i * P:(i + 1) * P, :]) for i in range(4)]
            vrow = ctile([1, 4 * (HD + 1)], "vrow", vrow_d[:, :])
            bq = ctile([P, 2], "bq", bq_d[:, :])
            bk = ctile([P, 2], "bk", bk_d[:, :])
            bm = ctile([P, 2], "bm", bm_d[:, :])
            b1 = ctile([P, 4], "b1", b1_d[:, :])
            b2 = ctile([P, 2], "b2", b2_d[:, :])
            ones_col = cpool.tile([1, P], F32, tag="ones")
            nc.gpsimd.memset(ones_col[:], 1.0)

            x_sb = [ppool.tile([P, NC], F32, tag=f"x{i}", name=f"x{i}") for i in range(2)]
            src_sb = [ppool.tile([P, M], F32, tag=f"s{i}", name=f"s{i}") for i in range(2)]
            for i in range(2):
                nc.sync.dma_start(x_sb[i][:], x_d[i * P:(i + 1) * P, :])
                nc.sync.dma_start(src_sb[i][:], src_d[i * P:(i + 1) * P, :])

            q_sb = [ppool.tile([P, NC], F32, tag=f"q{i}", name=f"q{i}") for i in range(2)]
            k_sb = [ppool.tile([P, M], F32, tag=f"k{i}", name=f"k{i}") for i in range(2)]
            vT_sb = [ppool.tile([P, 4 * (HD + 1)], F32, tag=f"v{i}", name=f"v{i}") for i in range(16)]
            msg_sb = [ppool.tile([P, NC], F32, tag=f"m{i}", name=f"m{i}") for i in range(2)]
            mm_sb = [ppool.tile([P, NC], F32, tag=f"mm{i}", name=f"mm{i}") for i in range(2)]
            h1_sb = [ppool.tile([P, NC], F32, tag=f"h{i}", name=f"h{i}") for i in range(4)]

            # ---- phase 1: q, k projections ----
            for cb in range(2):
                ps = psb.tile([P, 1024], F32, tag="big")
                for nh in range(2):
                    for dc in range(2):
                        nc.tensor.matmul(
                            ps[:, nh * 512:(nh + 1) * 512],
                            wqT[dc][:, cb * P:(cb + 1) * P],
                            x_sb[dc][:, nh * 512:(nh + 1) * 512],
                            start=(dc == 0), stop=(dc == 1),
                        )
                nc.scalar.activation(q_sb[cb][:], ps[:], AF.Identity,
                                     bias=bq[:, cb:cb + 1])
            for cb in range(2):
                for mh in range(2):
                    ps = psb.tile([P, 1024], F32, tag="big")
                    for ms in range(2):
                        for dc in range(2):
                            nc.tensor.matmul(
                                ps[:, ms * 512:(ms + 1) * 512],
                                wkT[dc][:, cb * P:(cb + 1) * P],
                                src_sb[dc][:, mh * 1024 + ms * 512:mh * 1024 + (ms + 1) * 512],
                                start=(dc == 0), stop=(dc == 1),
                            )
                    nc.scalar.activation(k_sb[cb][:, mh * 1024:(mh + 1) * 1024],
                                         ps[:], AF.Identity, bias=bk[:, cb:cb + 1])

            # ---- phase 1b: vT (head-major 65-col blocks, ones col for sumexp) ----
            W65 = 4 * (HD + 1)
            for mb in range(16):
                psv = psa.tile([P, W65], F32, tag="acc")
                for dc in range(2):
                    nc.tensor.matmul(psv[:], src_sb[dc][:, mb * P:(mb + 1) * P],
                                     wvT[dc][:], start=(dc == 0),
                                     stop=(zero_bias and dc == 1))
                if not zero_bias:
                    nc.tensor.matmul(psv[:], ones_col[0:1, :], vrow[0:1, :],
                                     start=False, stop=True)
                nc.scalar.activation(vT_sb[mb][:], psv[:], AF.Copy)
                if zero_bias:
                    base = vT_sb[mb][:, HD:HD + 1]
                    ones_ap = bass.AP(base.tensor, base.offset,
                                      [base.ap[0], [HD + 1, 4]])
                    nc.gpsimd.memset(ones_ap, 1.0)

            # ---- phase 2: attention, per 512-query window ----
            for ncw in range(2):
                nsl = slice(ncw * 512, (ncw + 1) * 512)
                ps_msg = [psa.tile([HD + 1, 512], F32, tag="acc", name="psmsg") for _ in range(4)]
                for mbq in range(4):
                    mbs = [4 * mbq + j for j in range(4)]
                    wm = wmpool.tile([P, 2048], MASK_DT, tag="wm")
                    for j, mb in enumerate(mbs):
                        nc.sync.dma_start(wm[:, j * 512:(j + 1) * 512],
                                          wm_d[mb * P:(mb + 1) * P, nsl])
                    for h in range(4):
                        cb, off = h // 2, 64 * (h % 2)
                        masked = mkpool.tile([P, 2048], F32, tag="mk")
                        for half in range(2):
                            ps_s = psb.tile([P, 1024], F32, tag="big", name="ps_s")
                            for j in range(2):
                                mb = mbs[2 * half + j]
                                nc.tensor.matmul(
                                    ps_s[:, j * 512:(j + 1) * 512],
                                    k_sb[cb][off:off + 64, mb * P:(mb + 1) * P],
                                    q_sb[cb][off:off + 64, nsl],
                                    start=True, stop=True)
                            nc.vector.tensor_mul(
                                masked[:, half * 1024:(half + 1) * 1024],
                                ps_s[:], wm[:, half * 1024:(half + 1) * 1024])
                        expt = expool.tile([P, 2048], EXP_DT, tag="ex")
                        nc.scalar.activation(expt[:], masked[:], AF.Exp)
                        for j, mb in enumerate(mbs):
                            nc.tensor.matmul(
                                ps_msg[h][:],
                                vT_sb[mb][:, h * (HD + 1):(h + 1) * (HD + 1)],
                                expt[:, j * 512:(j + 1) * 512],
                                start=(mbq == 0 and j == 0),
                                stop=(mbq == 3 and j == 3))
                # normalize: msg = umsg * (1/sumexp) broadcast over the 64 channels
                for h in range(4):
                    cb, off = h // 2, 64 * (h % 2)
                    recip = smpool.tile([1, 512], F32, tag="rc")
                    nc.vector.reciprocal(recip[:], ps_msg[h][HD:HD + 1, :])
                    rb = smpool.tile([64, 512], F32, tag="rb")
                    nc.sync.dma_start(
                        rb[:], recip[0:1, None, :].broadcast_to([1, 64, 512]))
                    nc.vector.tensor_mul(msg_sb[cb][off:off + 64, nsl],
                                         ps_msg[h][0:HD, :], rb[:])

                # ---- phase 3: merge + MLP for this window ----
                for cb in range(2):
                    ps = psb.tile([P, 512], F32, tag="big")
                    for cc in range(2):
                        nc.tensor.matmul(ps[:], wmT[cc][:, cb * P:(cb + 1) * P],
                                         msg_sb[cc][:, nsl],
                                         start=(cc == 0), stop=(cc == 1))
                    nc.scalar.activation(mm_sb[cb][:, nsl], ps[:], AF.Identity,
                                         bias=bm[:, cb:cb + 1])
                z = [x_sb[0], x_sb[1], mm_sb[0], mm_sb[1]]
                for c4 in range(4):
                    ps = psb.tile([P, 512], F32, tag="big")
                    for zc in range(4):
                        nc.tensor.matmul(ps[:], w1T[zc][:, c4 * P:(c4 + 1) * P],
                                         z[zc][:, nsl],
                                         start=(zc == 0), stop=(zc == 3))
                    nc.scalar.activation(h1_sb[c4][:, nsl], ps[:], AF.Relu,
                                         bias=b1[:, c4:c4 + 1])
                for cb in range(2):
                    ps = psb.tile([P, 512], F32, tag="big")
                    for hc in range(4):
                        nc.tensor.matmul(ps[:], w2T[hc][:, cb * P:(cb + 1) * P],
                                         h1_sb[hc][:, nsl],
                                         start=(hc == 0), stop=(hc == 3))
                    outt = otpool.tile([P, 512], F32, tag="ot")
                    nc.vector.tensor_scalar_add(outt[:], ps[:], b2[:, cb:cb + 1])
                    nc.sync.dma_start(out_d[cb * P:(cb + 1) * P, nsl], outt[:])

    nc.compile()
    return nc


def host_prep(x, source, weight, mask, Wq, bq, Wk, bk, Wv, bv, Wm, bm,
              W1, b1, g1, be1, W2, b2):
    """Build the per-core input maps (numpy only)."""
    f = np.float32
    perm = np.arange(D).reshape(HD, H).T.reshape(-1)  # perm[h*64+hd] = hd*4+h

    wqT = np.ascontiguousarray(Wq[perm].T, dtype=f)
    wkT = np.ascontiguousarray(Wk[perm].T, dtype=f)
    wvT_p = Wv[perm].T  # [d, c_p]
    wvT = np.zeros((D, 4 * (HD + 1)), f)
    vrow = np.zeros((1, 4 * (HD + 1)), f)
    bv_p = bv[perm]
    for h in range(H):
        wvT[:, h * (HD + 1):h * (HD + 1) + HD] = wvT_p[:, h * HD:(h + 1) * HD]
        vrow[0, h * (HD + 1):h * (HD + 1) + HD] = bv_p[h * HD:(h + 1) * HD]
        vrow[0, h * (HD + 1) + HD] = 1.0
    wmT = np.ascontiguousarray(Wm[:, perm].T, dtype=f)
    gs = (g1 / np.sqrt(1.0 + 0.001)).astype(f)
    w1T = np.ascontiguousarray((W1 * gs[:, None]).T, dtype=f)
    b1p = (gs * b1 + be1).astype(f)
    w2T = np.ascontiguousarray(W2.T, dtype=f)

    shared = {
        "wqT": wqT, "wkT": wkT, "wvT": wvT, "vrow": vrow, "wmT": wmT,
        "w1T": w1T, "w2T": w2T,
        "bq2": np.ascontiguousarray(bq[perm].reshape(2, P).T, dtype=f),
        "bk2": np.ascontiguousarray(bk[perm].reshape(2, P).T, dtype=f),
        "bm2": np.ascontiguousarray(bm.reshape(2, P).T, dtype=f),
        "b1p4": np.ascontiguousarray(b1p.reshape(4, P).T, dtype=f),
        "b22": np.ascontiguousarray(b2.reshape(2, P).T, dtype=f),
    }

    in_maps = []
    for core in range(N_CORES):
        b, s = core // 2, core % 2
        n0 = s * NC
        wmask_b = (mask[b].T * (weight[b] / 8.0)[:, None])[:, n0:n0 + NC]
        m = dict(shared)
        m["x_sl"] = np.ascontiguousarray(x[b][:, n0:n0 + NC], dtype=f)
        m["src"] = np.ascontiguousarray(source[b], dtype=f)
        m["wmask"] = np.ascontiguousarray(wmask_b, dtype=np.float32 if MASK_DT == F32 else np.float32)
        in_maps.append(m)
    return in_maps


def kernel(**inputs):
    zb = all(not np.any(inputs[k]) for k in ("bq", "bk", "bv", "bm", "b2")) \
        and not np.any(inputs["b1"] * inputs["g1"] + inputs["be1"])
    key = ("nc", zb)
    if key not in _cached:
        _cached[key] = build_program(zero_bias=zb)
    nc = _cached[key]
    in_maps = host_prep(**inputs)
    res = run_bass_kernel_spmd(nc, in_maps, list(range(N_CORES)))
    out = np.zeros((B, D, N), np.float32)
    for core in range(N_CORES):
        b, s = core // 2, core % 2
        out[b][:, s * NC:(s + 1) * NC] = res.results[core]["out"]
    return out
